# revision 13
# baseline (speedup 1.0000x reference)
"""Trainium2 Bass kernel for the two-stage DAN/MoVe attention module.

Computation (per batch b):
  Stage 1:  S  = skT.T @ q1 / sqrt(C);  P  = softmax_k(S);   newV^T = P.T-contracted with svT
            (computed as newVT[q, vc] = sum_k P[k, q] * svT[k, vc])
  Stage 2:  S2 = mK.T @ qq / sqrt(C);   P2 = softmax_k2(S2); out^T[q2, vc] = sum_k2 P2[k2, q2] * nvtn[k2, vc]

Sharding: 8 cores = 2 batches x 4 lanes. Stage 1 splits the 1600 query
columns 4 ways (400 each); stage 2 splits the 14400 frame-query columns
4 ways (3712-wide windows, 3600 owned). Two SPMD launches; the host
passes stage-1 results (unnormalized newVT + column sums) into stage 2,
where normalization happens on-device via per-partition reciprocal scales.

Matmuls run in float32r (single-pass fp32 PE mode, ~1.5e-4 rel err,
4x faster than fp32). Softmax skips the max-subtraction (scores are
~N(0,1); exp cannot overflow) so only exp + column sums are needed;
column sums come from ones-matmuls on the tensor engine.
"""

import math

import numpy as np

import concourse.bass as bass
import concourse.tile as tile
from concourse import bacc, mybir
from concourse.bass_utils import run_bass_kernel_spmd

F32 = mybir.dt.float32
F32R = mybir.dt.float32r
EXP = mybir.ActivationFunctionType.Exp

B, FRAME, SFRAME, C, VC, H, W = 2, 9, 15, 128, 512, 40, 40
HW = H * W                      # 1600
MID = FRAME // 2                # 4
WK = SFRAME * HW                # 24000 support keys
NKT = (WK + 127) // 128         # 188 key tiles (last = 64 rows)
Q2 = FRAME * HW                 # 14400 stage-2 query columns per batch
NK2T = (HW + 127) // 128        # 13 stage-2 key tiles (last = 64 rows)

L1_COLS = HW // 4               # 400 owned stage-1 columns per lane
L1_SUBS = [(0, 128), (128, 128), (256, 128), (384, 16)]
L2_WIN = 3712                   # 29 * 128, per-lane stage-2 window
L2_OWN = Q2 // 4                # 3600 owned columns
L2_CHUNKS = [512] * 7 + [128]
INV_SQRT_C = 1.0 / math.sqrt(C)

_cache = {}


def _build_stage1():
    nc = bacc.Bacc("TRN2", target_bir_lowering=False, debug=False, num_devices=8)
    skt = nc.dram_tensor("skt", [C, WK], F32R, kind="ExternalInput").ap()
    svt = nc.dram_tensor("svt", [WK, VC], F32R, kind="ExternalInput").ap()
    q1 = nc.dram_tensor("q1", [C, L1_COLS], F32R, kind="ExternalInput").ap()
    ones = nc.dram_tensor("ones", [128, 2], F32R, kind="ExternalInput").ap()
    mun = nc.dram_tensor("mun", [L1_COLS, VC], F32, kind="ExternalOutput").ap()
    csum = nc.dram_tensor("csum", [2, L1_COLS], F32, kind="ExternalOutput").ap()

    with tile.TileContext(nc) as tc:
        with (
            tc.tile_pool(name="const", bufs=1) as cpool,
            tc.tile_pool(name="skt", bufs=1) as skpool,
            tc.tile_pool(name="svt", bufs=4) as svpool,
            tc.tile_pool(name="p", bufs=3) as ppool,
            tc.tile_pool(name="out", bufs=5) as opool,
            tc.tile_pool(name="ps_s", bufs=2, space="PSUM") as ps_s,
            tc.tile_pool(name="ps_m", bufs=1, space="PSUM") as ps_m,
            tc.tile_pool(name="ps_c", bufs=1, space="PSUM") as ps_c,
        ):
            q1_t = cpool.tile([C, L1_COLS], F32R)
            nc.sync.dma_start(q1_t[:], q1[:])
            ones_t = cpool.tile([128, 2], F32R)
            nc.sync.dma_start(ones_t[:], ones[:])

            # whole skT resident; geometric chunk sizes so the first key
            # tiles land within a couple of microseconds of launch
            skt_t = skpool.tile([C, WK], F32R)
            o = 0
            for w in (256, 768, 1024, 2048, 4096, 4096, 4096, 7616):
                nc.sync.dma_start(skt_t[:, o:o + w], skt[:, o:o + w])
                o += w
            assert o == WK

            m_ps = [ps_m.tile([128, VC], F32, name=f"m_ps{s}", tag=f"m_ps{s}")
                    for s in range(len(L1_SUBS))]
            c_ps = ps_c.tile([2, L1_COLS], F32)

            for kt in range(NKT):
                kk = min(128, WK - kt * 128)
                r0 = kt * 128
                sv_t = svpool.tile([128, VC], F32R)
                nc.sync.dma_start(sv_t[:kk, :], svt[r0:r0 + kk, :])
                s_ps = ps_s.tile([128, L1_COLS], F32)
                nc.tensor.matmul(s_ps[:kk, :], skt_t[:, r0:r0 + kk], q1_t[:],
                                 start=True, stop=True)
                p_t = ppool.tile([128, L1_COLS], F32R)
                nc.scalar.activation(p_t[:kk, :], s_ps[:kk, :], EXP,
                                     scale=INV_SQRT_C)
                nc.tensor.matmul(c_ps[:], ones_t[:kk, :], p_t[:kk, :],
                                 start=(kt == 0), stop=(kt == NKT - 1))
                for s, (o, w) in enumerate(L1_SUBS):
                    nc.tensor.matmul(m_ps[s][:w, :], p_t[:kk, o:o + w],
                                     sv_t[:kk, :],
                                     start=(kt == 0), stop=(kt == NKT - 1))

            for s, (o, w) in enumerate(L1_SUBS):
                m_sb = opool.tile([128, VC], F32)
                nc.vector.tensor_copy(m_sb[:w, :], m_ps[s][:w, :])
                nc.sync.dma_start(mun[o:o + w, :], m_sb[:w, :])
            c_sb = opool.tile([2, L1_COLS], F32)
            nc.vector.tensor_copy(c_sb[:], c_ps[:])
            nc.sync.dma_start(csum[:], c_sb[:])
    nc.compile()
    return nc


def _build_stage2():
    nc = bacc.Bacc("TRN2", target_bir_lowering=False, debug=False, num_devices=8)
    mk = nc.dram_tensor("mk", [C, HW], F32R, kind="ExternalInput").ap()
    qq = nc.dram_tensor("qq", [C, L2_WIN], F32R, kind="ExternalInput").ap()
    nvt = nc.dram_tensor("nvt", [HW, VC], F32R, kind="ExternalInput").ap()
    cs1 = nc.dram_tensor("cs1", [HW, 1], F32, kind="ExternalInput").ap()
    ones = nc.dram_tensor("ones", [128, 2], F32R, kind="ExternalInput").ap()
    out = nc.dram_tensor("out", [L2_WIN, VC], F32, kind="ExternalOutput").ap()

    with tile.TileContext(nc) as tc:
        with (
            tc.tile_pool(name="const", bufs=1) as cpool,
            tc.tile_pool(name="nvt", bufs=1) as nvpool,
            tc.tile_pool(name="small", bufs=4) as smpool,
            tc.tile_pool(name="p2", bufs=26) as p2pool,
            tc.tile_pool(name="ob", bufs=3) as obpool,
            tc.tile_pool(name="ps_s", bufs=2, space="PSUM") as ps_s,
            tc.tile_pool(name="ps_o", bufs=2, space="PSUM") as ps_o,
            tc.tile_pool(name="ps_c", bufs=2, space="PSUM") as ps_c,
        ):
            mk_t = cpool.tile([C, HW], F32R)
            nc.sync.dma_start(mk_t[:], mk[:])
            ones_t = cpool.tile([128, 2], F32R)
            nc.sync.dma_start(ones_t[:], ones[:])
            qq_t = cpool.tile([C, L2_WIN], F32R)
            for o in range(0, L2_WIN, 1856):
                nc.sync.dma_start(qq_t[:, o:o + 1856], qq[:, o:o + 1856])

            # load newVT tiles, normalize by stage-1 column sums (per-partition)
            nvtn = []
            for t in range(NK2T):
                kk = min(128, HW - t * 128)
                r0 = t * 128
                raw = smpool.tile([128, VC], F32R, tag="nvraw")
                nc.sync.dma_start(raw[:kk, :], nvt[r0:r0 + kk, :])
                cs_t = smpool.tile([128, 1], F32, tag="cs")
                nc.sync.dma_start(cs_t[:kk, :], cs1[r0:r0 + kk, :])
                rc_t = smpool.tile([128, 1], F32, tag="rc")
                nc.vector.reciprocal(rc_t[:kk, :], cs_t[:kk, :])
                nrm = nvpool.tile([128, VC], F32R, tag=f"nvtn{t}")
                nc.vector.tensor_scalar_mul(nrm[:kk, :], raw[:kk, :], rc_t[:kk, 0:1])
                nvtn.append(nrm)

            col = 0
            for chunk in L2_CHUNKS:
                p2 = []
                for t in range(NK2T):
                    kk = min(128, HW - t * 128)
                    s_ps = ps_s.tile([128, 512], F32)
                    nc.tensor.matmul(s_ps[:kk, :chunk],
                                     mk_t[:, t * 128:t * 128 + kk],
                                     qq_t[:, col:col + chunk],
                                     start=True, stop=True)
                    p_t = p2pool.tile([128, 512], F32R, tag="p2")
                    nc.scalar.activation(p_t[:kk, :chunk], s_ps[:kk, :chunk],
                                         EXP, scale=INV_SQRT_C)
                    p2.append(p_t)
                for sub in range(chunk // 128):
                    so = sub * 128
                    o_ps = ps_o.tile([128, VC], F32)
                    c_ps = ps_c.tile([128, 2], F32)
                    for t in range(NK2T):
                        kk = min(128, HW - t * 128)
                        # csum first: its (tiny) matmul loads the P2 weights,
                        # the long out2 matmul then reuses them (walrus LW
                        # dedupe) and hides the next iteration's weight load.
                        nc.tensor.matmul(c_ps[:], p2[t][:kk, so:so + 128],
                                         ones_t[:kk, :],
                                         start=(t == 0), stop=(t == NK2T - 1))
                        nc.tensor.matmul(o_ps[:], p2[t][:kk, so:so + 128],
                                         nvtn[t][:kk, :],
                                         start=(t == 0), stop=(t == NK2T - 1))
                    rc = smpool.tile([128, 1], F32, tag="rc2")
                    nc.vector.reciprocal(rc[:], c_ps[:, 0:1])
                    ob = obpool.tile([128, VC], F32)
                    nc.vector.tensor_scalar_mul(ob[:], o_ps[:], rc[:, 0:1])
                    nc.sync.dma_start(out[col + so:col + so + 128, :], ob[:])
                col += chunk
    nc.compile()
    return nc


def _run_with_retry(build_key, builder, in_maps):
    """Run a launch; on a transient device failure retry, rebuilding the
    program (fresh jit identity) on the second failure."""
    last = None
    for attempt in range(3):
        if build_key not in _cache:
            _cache[build_key] = builder()
        try:
            return run_bass_kernel_spmd(_cache[build_key], in_maps,
                                        list(range(8)))
        except Exception as e:  # device wedge / transient axon failure
            last = e
            import time
            time.sleep(3.0)
            if attempt >= 1:
                _cache.pop(build_key, None)
    raise last


def kernel(query_q, query_k, support_k, support_v):
    query_q = np.ascontiguousarray(query_q, dtype=np.float32)
    query_k = np.ascontiguousarray(query_k, dtype=np.float32)
    support_k = np.ascontiguousarray(support_k, dtype=np.float32)
    support_v = np.ascontiguousarray(support_v, dtype=np.float32)

    ones = np.ones((128, 2), np.float32)

    # ---- host layout prep ----
    # skT[b]: [C, SF*HW], svT[b]: [SF*HW, VC], q1[b]: [C, HW]
    skt = support_k.transpose(0, 2, 1, 3, 4).reshape(B, C, WK)
    svt = support_v.transpose(0, 1, 3, 4, 2).reshape(B, WK, VC)
    q1 = query_q[:, MID].reshape(B, C, HW)
    l1_maps = []
    for core in range(8):
        b, lane = divmod(core, 4)
        l1_maps.append({
            "skt": skt[b],
            "svt": svt[b],
            "q1": np.ascontiguousarray(q1[b][:, lane * L1_COLS:(lane + 1) * L1_COLS]),
            "ones": ones,
        })
    res1 = _run_with_retry("l1", _build_stage1, l1_maps)
    r1 = res1.results

    nvt = np.empty((B, HW, VC), np.float32)
    cs1 = np.empty((B, HW, 1), np.float32)
    for core in range(8):
        b, lane = divmod(core, 4)
        sl = slice(lane * L1_COLS, (lane + 1) * L1_COLS)
        nvt[b][sl] = r1[core]["mun"]
        cs1[b][sl, 0] = r1[core]["csum"][0]

    # ---- stage 2 ----
    mk = query_k[:, MID].reshape(B, C, HW)
    qq = query_q.transpose(0, 2, 1, 3, 4).reshape(B, C, Q2)
    wins = [0, L2_OWN, 2 * L2_OWN, Q2 - L2_WIN]
    l2_maps = []
    for core in range(8):
        b, lane = divmod(core, 4)
        w = wins[lane]
        l2_maps.append({
            "mk": mk[b],
            "qq": np.ascontiguousarray(qq[b][:, w:w + L2_WIN]),
            "nvt": nvt[b],
            "cs1": cs1[b],
            "ones": ones,
        })
    res2 = _run_with_retry("l2", _build_stage2, l2_maps)
    r2 = res2.results
    _cache["last_exec_ns"] = [res1.exec_time_ns, res2.exec_time_ns]

    outT = np.empty((B, Q2, VC), np.float32)
    for core in range(8):
        b, lane = divmod(core, 4)
        w = wins[lane]
        lo = lane * L2_OWN - w
        outT[b][lane * L2_OWN:(lane + 1) * L2_OWN] = r2[core]["out"][lo:lo + L2_OWN]

    # outT[b][q2, vc], q2 = f*HW + h*W + w  ->  [B, F, VC, H, W]
    return np.ascontiguousarray(
        outT.reshape(B, FRAME, H, W, VC).transpose(0, 1, 4, 2, 3))


# revision 14
# speedup vs baseline: 1.1091x; 1.1091x over previous
"""Trainium2 Bass kernel for the two-stage DAN/MoVe attention module.

Computation (per batch b, C=128 channels):
  Stage 1:  S  = skT.T @ q1 / sqrt(C);  P  = softmax_k(S);  newV = sv @ P
  Stage 2:  S2 = mK.T @ qq / sqrt(C);   P2 = softmax_k2(S2); out = newV @ P2

Sharding: 8 cores = 2 batches x 4 lanes. Stage 1 splits the 1600 query
columns 4 ways (400 each); stage 2 splits the 14400 frame-query columns
4 ways (3712-wide windows, 3600 owned). Two SPMD launches; the host
transposes stage-1 results between launches.

All big matmuls run in float32r (single-pass fp32 PE mode, ~1.5e-4 rel
err, 4x faster than fp32) with the value/key matrices as the stationary
operand and exp(S) as the long moving operand, so weight loads hide
under the previous matmul's stream. Softmax skips max-subtraction
(scores are ~N(0,1); exp cannot overflow). Column sums fall out of two
ones-columns prepended to the value matrices (an M=2 matmul per key
tile); normalization happens on-device via reciprocal + per-partition
scaling (stage 1 sums applied in stage 2) or partition-broadcast
multiply (stage 2 sums).
"""

import math
import time

import numpy as np

import concourse.bass as bass
import concourse.tile as tile
from concourse import bacc, mybir
from concourse.bass_utils import run_bass_kernel_spmd

F32 = mybir.dt.float32
F32R = mybir.dt.float32r
EXP = mybir.ActivationFunctionType.Exp

B, FRAME, SFRAME, C, VC, H, W = 2, 9, 15, 128, 512, 40, 40
HW = H * W                      # 1600
MID = FRAME // 2                # 4
WK = SFRAME * HW                # 24000 support keys
NKT = (WK + 127) // 128         # 188 key tiles (last = 64 rows)
Q2 = FRAME * HW                 # 14400 stage-2 query columns per batch
NK2T = (HW + 127) // 128        # 13 stage-2 key tiles (last = 64 rows)
VE = VC + 2                     # value matrices carry 2 ones-columns

L1_COLS = HW // 4               # 400 owned stage-1 columns per lane
L2_WIN = 3712                   # 29 * 128, per-lane stage-2 window
L2_OWN = Q2 // 4                # 3600 owned columns
L2_CHUNKS = [512] * 7 + [128]
INV_SQRT_C = 1.0 / math.sqrt(C)

_cache = {}


def _build_stage1():
    nc = bacc.Bacc("TRN2", target_bir_lowering=False, debug=False, num_devices=8)
    skt = nc.dram_tensor("skt", [C, WK], F32R, kind="ExternalInput").ap()
    svte = nc.dram_tensor("svte", [WK, VE], F32R, kind="ExternalInput").ap()
    q1 = nc.dram_tensor("q1", [C, L1_COLS], F32R, kind="ExternalInput").ap()
    nv = nc.dram_tensor("nv", [VC, L1_COLS], F32, kind="ExternalOutput").ap()
    csum = nc.dram_tensor("csum", [2, L1_COLS], F32, kind="ExternalOutput").ap()

    with tile.TileContext(nc) as tc:
        with (
            tc.tile_pool(name="const", bufs=1) as cpool,
            tc.tile_pool(name="skt", bufs=1) as skpool,
            tc.tile_pool(name="svt", bufs=4) as svpool,
            tc.tile_pool(name="p", bufs=3) as ppool,
            tc.tile_pool(name="out", bufs=5) as opool,
            tc.tile_pool(name="ps_s", bufs=2, space="PSUM") as ps_s,
            tc.tile_pool(name="ps_m", bufs=1, space="PSUM") as ps_m,
            tc.tile_pool(name="ps_c", bufs=1, space="PSUM") as ps_c,
        ):
            q1_t = cpool.tile([C, L1_COLS], F32R)
            nc.sync.dma_start(q1_t[:], q1[:])

            # skT resident via the gpsimd (SWDGE) queues so the per-tile
            # svte stream on the sync (HWDGE) queues is never stuck
            # behind these bulk loads; small leading chunks unblock the
            # first matmuls fast.
            skt_t = skpool.tile([C, WK], F32R)
            o = 0
            for w in (256, 768, 1024, 2048, 4096, 4096, 4096, 7616):
                nc.gpsimd.dma_start(skt_t[:, o:o + w], skt[:, o:o + w])
                o += w
            assert o == WK

            m_ps = [ps_m.tile([128, L1_COLS], F32, name=f"m_ps{s}", tag=f"m_ps{s}")
                    for s in range(4)]
            c_ps = ps_c.tile([2, L1_COLS], F32)

            for kt in range(NKT):
                kk = min(128, WK - kt * 128)
                r0 = kt * 128
                sv_t = svpool.tile([128, VE], F32R)
                nc.sync.dma_start(sv_t[:kk, :], svte[r0:r0 + kk, :])
                s_ps = ps_s.tile([128, L1_COLS], F32)
                nc.tensor.matmul(s_ps[:kk, :], skt_t[:, r0:r0 + kk], q1_t[:],
                                 start=True, stop=True)
                p_t = ppool.tile([128, L1_COLS], F32R)
                nc.scalar.activation(p_t[:kk, :], s_ps[:kk, :], EXP,
                                     scale=INV_SQRT_C)
                nc.tensor.matmul(c_ps[:], sv_t[:kk, 0:2], p_t[:kk, :],
                                 start=(kt == 0), stop=(kt == NKT - 1))
                for s in range(4):
                    nc.tensor.matmul(m_ps[s][:], sv_t[:kk, 2 + 128 * s:2 + 128 * (s + 1)],
                                     p_t[:kk, :],
                                     start=(kt == 0), stop=(kt == NKT - 1))

            for s in range(4):
                m_sb = opool.tile([128, L1_COLS], F32, name=f"m_sb{s}", tag="m_sb")
                nc.vector.tensor_copy(m_sb[:], m_ps[s][:])
                nc.sync.dma_start(nv[128 * s:128 * (s + 1), :], m_sb[:])
            c_sb = opool.tile([2, L1_COLS], F32)
            nc.vector.tensor_copy(c_sb[:], c_ps[:])
            nc.sync.dma_start(csum[:], c_sb[:])
    nc.compile()
    return nc


def _build_stage2():
    nc = bacc.Bacc("TRN2", target_bir_lowering=False, debug=False, num_devices=8)
    mk = nc.dram_tensor("mk", [C, HW], F32R, kind="ExternalInput").ap()
    qq = nc.dram_tensor("qq", [C, L2_WIN], F32R, kind="ExternalInput").ap()
    nvte = nc.dram_tensor("nvte", [HW, VE], F32R, kind="ExternalInput").ap()
    cs1 = nc.dram_tensor("cs1", [HW, 1], F32, kind="ExternalInput").ap()
    out = nc.dram_tensor("out", [VC, L2_WIN], F32, kind="ExternalOutput").ap()

    with tile.TileContext(nc) as tc:
        with (
            tc.tile_pool(name="const", bufs=1) as cpool,
            tc.tile_pool(name="nvt", bufs=1) as nvpool,
            tc.tile_pool(name="small", bufs=4) as smpool,
            tc.tile_pool(name="p2", bufs=26) as p2pool,
            tc.tile_pool(name="ob", bufs=6) as obpool,
            tc.tile_pool(name="ps_s", bufs=2, space="PSUM") as ps_s,
            tc.tile_pool(name="ps_o", bufs=1, space="PSUM") as ps_o,
            tc.tile_pool(name="ps_c", bufs=2, space="PSUM") as ps_c,
        ):
            mk_t = cpool.tile([C, HW], F32R)
            nc.gpsimd.dma_start(mk_t[:], mk[:])
            qq_t = cpool.tile([C, L2_WIN], F32R)
            nc.sync.dma_start(qq_t[:, 0:512], qq[:, 0:512])
            nc.gpsimd.dma_start(qq_t[:, 512:L2_WIN], qq[:, 512:L2_WIN])

            # load newV tiles; normalize the value part (cols 2:) by the
            # stage-1 column sums, keep the ones-columns unscaled so they
            # still produce stage-2 column sums.
            nvtn = []
            for t in range(NK2T):
                kk = min(128, HW - t * 128)
                r0 = t * 128
                raw = smpool.tile([128, VE], F32R, tag="nvraw")
                nc.sync.dma_start(raw[:kk, :], nvte[r0:r0 + kk, :])
                cs_t = smpool.tile([128, 1], F32, tag="cs")
                nc.sync.dma_start(cs_t[:kk, :], cs1[r0:r0 + kk, :])
                rc_t = smpool.tile([128, 1], F32, tag="rc")
                nc.vector.reciprocal(rc_t[:kk, :], cs_t[:kk, :])
                nrm = nvpool.tile([128, VE], F32R, tag=f"nvtn{t}", name=f"nvtn{t}")
                nc.vector.tensor_scalar_mul(nrm[:kk, 2:], raw[:kk, 2:],
                                            rc_t[:kk, 0:1])
                nc.vector.tensor_copy(nrm[:kk, 0:2], raw[:kk, 0:2])
                nvtn.append(nrm)

            col = 0
            for chunk in L2_CHUNKS:
                p2 = []
                for t in range(NK2T):
                    kk = min(128, HW - t * 128)
                    s_ps = ps_s.tile([128, 512], F32)
                    nc.tensor.matmul(s_ps[:kk, :chunk],
                                     mk_t[:, t * 128:t * 128 + kk],
                                     qq_t[:, col:col + chunk],
                                     start=True, stop=True)
                    p_t = p2pool.tile([128, 512], F32R, tag="p2")
                    nc.scalar.activation(p_t[:kk, :chunk], s_ps[:kk, :chunk],
                                         EXP, scale=INV_SQRT_C)
                    p2.append(p_t)

                o_ps = [ps_o.tile([128, 512], F32, name=f"o_ps{v}", tag=f"o_ps{v}")
                        for v in range(4)]
                c_ps = ps_c.tile([2, 512], F32)
                for t in range(NK2T):
                    kk = min(128, HW - t * 128)
                    nc.tensor.matmul(c_ps[:, :chunk], nvtn[t][:kk, 0:2],
                                     p2[t][:kk, :chunk],
                                     start=(t == 0), stop=(t == NK2T - 1))
                    for v in range(4):
                        nc.tensor.matmul(o_ps[v][:, :chunk],
                                         nvtn[t][:kk, 2 + 128 * v:2 + 128 * (v + 1)],
                                         p2[t][:kk, :chunk],
                                         start=(t == 0), stop=(t == NK2T - 1))

                rc = smpool.tile([1, 512], F32, tag="rc2")
                nc.vector.reciprocal(rc[:, :chunk], c_ps[0:1, :chunk])
                bc = smpool.tile([128, 512], F32, tag="bc")
                nc.gpsimd.partition_broadcast(bc[:, :chunk], rc[:1, :chunk])
                for v in range(4):
                    ob = obpool.tile([128, 512], F32, tag="ob")
                    nc.vector.tensor_mul(ob[:, :chunk], o_ps[v][:, :chunk],
                                         bc[:, :chunk])
                    nc.sync.dma_start(out[128 * v:128 * (v + 1), col:col + chunk],
                                      ob[:, :chunk])
                col += chunk
    nc.compile()
    return nc


def _run_with_retry(build_key, builder, in_maps):
    """Run a launch; on a transient device failure retry, rebuilding the
    program (fresh jit identity) on the second failure."""
    last = None
    for attempt in range(3):
        if build_key not in _cache:
            _cache[build_key] = builder()
        try:
            return run_bass_kernel_spmd(_cache[build_key], in_maps,
                                        list(range(8)))
        except Exception as e:  # device wedge / transient axon failure
            last = e
            time.sleep(3.0)
            if attempt >= 1:
                _cache.pop(build_key, None)
    raise last


def kernel(query_q, query_k, support_k, support_v):
    query_q = np.ascontiguousarray(query_q, dtype=np.float32)
    query_k = np.ascontiguousarray(query_k, dtype=np.float32)
    support_k = np.ascontiguousarray(support_k, dtype=np.float32)
    support_v = np.ascontiguousarray(support_v, dtype=np.float32)

    # ---- host layout prep ----
    # skT[b]: [C, SF*HW];  svte[b]: [SF*HW, 2+VC] (2 ones-cols | sv.T)
    skt = support_k.transpose(0, 2, 1, 3, 4).reshape(B, C, WK)
    svte = np.empty((B, WK, VE), np.float32)
    svte[:, :, :2] = 1.0
    svte[:, :, 2:] = support_v.transpose(0, 1, 3, 4, 2).reshape(B, WK, VC)
    q1 = query_q[:, MID].reshape(B, C, HW)
    l1_maps = []
    for core in range(8):
        b, lane = divmod(core, 4)
        l1_maps.append({
            "skt": skt[b],
            "svte": svte[b],
            "q1": np.ascontiguousarray(q1[b][:, lane * L1_COLS:(lane + 1) * L1_COLS]),
        })
    res1 = _run_with_retry("l1", _build_stage1, l1_maps)
    r1 = res1.results

    # assemble newV^T (+ ones cols) and stage-1 column sums per batch
    nvte = np.empty((B, HW, VE), np.float32)
    nvte[:, :, :2] = 1.0
    cs1 = np.empty((B, HW, 1), np.float32)
    for core in range(8):
        b, lane = divmod(core, 4)
        sl = slice(lane * L1_COLS, (lane + 1) * L1_COLS)
        nvte[b][sl, 2:] = r1[core]["nv"].T
        cs1[b][sl, 0] = r1[core]["csum"][0]

    # ---- stage 2 ----
    mk = query_k[:, MID].reshape(B, C, HW)
    qq = query_q.transpose(0, 2, 1, 3, 4).reshape(B, C, Q2)
    wins = [0, L2_OWN, 2 * L2_OWN, Q2 - L2_WIN]
    l2_maps = []
    for core in range(8):
        b, lane = divmod(core, 4)
        w = wins[lane]
        l2_maps.append({
            "mk": mk[b],
            "qq": np.ascontiguousarray(qq[b][:, w:w + L2_WIN]),
            "nvte": nvte[b],
            "cs1": cs1[b],
        })
    res2 = _run_with_retry("l2", _build_stage2, l2_maps)
    r2 = res2.results
    _cache["last_exec_ns"] = [res1.exec_time_ns, res2.exec_time_ns]

    outv = np.empty((B, VC, Q2), np.float32)
    for core in range(8):
        b, lane = divmod(core, 4)
        w = wins[lane]
        lo = lane * L2_OWN - w
        outv[b][:, lane * L2_OWN:(lane + 1) * L2_OWN] = \
            r2[core]["out"][:, lo:lo + L2_OWN]

    # outv[b][vc, q2], q2 = f*HW + h*W + w  ->  [B, F, VC, H, W]
    return np.ascontiguousarray(
        outv.reshape(B, VC, FRAME, H, W).transpose(0, 2, 1, 3, 4))


# revision 24
# speedup vs baseline: 1.1796x; 1.0635x over previous
"""Trainium2 Bass kernel for the two-stage DAN/MoVe attention module.

Computation (per batch b, C=128 channels):
  Stage 1:  S  = skT.T @ q1 / sqrt(C);  P  = softmax_k(S);  newV = sv @ P
  Stage 2:  S2 = mK.T @ qq / sqrt(C);   P2 = softmax_k2(S2); out = newV @ P2

Sharding: 8 cores = 2 batches x 4 lanes. Stage 1 splits the 1600 query
columns 4 ways (400 each); stage 2 splits the 14400 frame-query columns
4 ways (3712-wide windows, 3600 owned). Two SPMD launches; the host
transposes stage-1 results between launches.

All big matmuls run in float32r (single-pass fp32 PE mode, ~1.5e-4 rel
err, 4x faster than fp32) with the value/key matrices as the stationary
operand and exp(S) as the long moving operand, so weight loads hide
under the previous matmul's stream. Softmax skips max-subtraction
(scores are ~N(0,1); exp cannot overflow). Column sums fall out of two
ones-columns prepended to the value matrices (an M=2 matmul per key
tile); normalization happens on-device via reciprocal + per-partition
scaling (stage 1 sums applied in stage 2) or partition-broadcast
multiply (stage 2 sums).
"""

import math
import time

import numpy as np

import concourse.bass as bass
import concourse.tile as tile
from concourse import bacc, mybir
from concourse.bass_utils import run_bass_kernel_spmd

F32 = mybir.dt.float32
F32R = mybir.dt.float32r
EXP = mybir.ActivationFunctionType.Exp

B, FRAME, SFRAME, C, VC, H, W = 2, 9, 15, 128, 512, 40, 40
HW = H * W                      # 1600
MID = FRAME // 2                # 4
WK = SFRAME * HW                # 24000 support keys
NKT = (WK + 127) // 128         # 188 key tiles (last = 64 rows)
Q2 = FRAME * HW                 # 14400 stage-2 query columns per batch
NK2T = (HW + 127) // 128        # 13 stage-2 key tiles (last = 64 rows)
VE = VC + 2                     # value matrices carry 2 ones-columns

L1_COLS = HW // 4               # 400 owned stage-1 columns per lane
L2_WIN = 3712                   # 29 * 128, per-lane stage-2 window
L2_OWN = Q2 // 4                # 3600 owned columns
L2_CHUNKS = [512] * 7 + [128]
INV_SQRT_C = 1.0 / math.sqrt(C)

_cache = {}


def _build_stage1():
    nc = bacc.Bacc("TRN2", target_bir_lowering=False, debug=False, num_devices=8)
    skt = nc.dram_tensor("skt", [C, WK], F32R, kind="ExternalInput").ap()
    svte = nc.dram_tensor("svte", [WK, VE], F32R, kind="ExternalInput").ap()
    q1 = nc.dram_tensor("q1", [C, L1_COLS], F32R, kind="ExternalInput").ap()
    nv = nc.dram_tensor("nv", [VC, L1_COLS], F32, kind="ExternalOutput").ap()
    csum = nc.dram_tensor("csum", [2, L1_COLS], F32, kind="ExternalOutput").ap()

    with tile.TileContext(nc) as tc:
        with (
            tc.tile_pool(name="const", bufs=1) as cpool,
            tc.tile_pool(name="skt", bufs=1) as skpool,
            tc.tile_pool(name="svt", bufs=10) as svpool,
            tc.tile_pool(name="p", bufs=6) as ppool,
            tc.tile_pool(name="pacc", bufs=3) as paccpool,
            tc.tile_pool(name="out", bufs=5) as opool,
            tc.tile_pool(name="ps_s", bufs=2, space="PSUM") as ps_s,
            tc.tile_pool(name="ps_m", bufs=1, space="PSUM") as ps_m,
            tc.tile_pool(name="ps_c", bufs=1, space="PSUM") as ps_c,
        ):
            q1_t = cpool.tile([C, L1_COLS], F32R)
            nc.sync.dma_start(q1_t[:], q1[:])

            # skT resident via the gpsimd (SWDGE) queues so the per-tile
            # svte stream on the sync (HWDGE) queues is never stuck
            # behind these bulk loads; small leading chunks unblock the
            # first matmuls fast.
            skt_t = skpool.tile([C, WK], F32R)
            o = 0
            for w in (128, 128, 256, 256, 512, 512, 1024, 1024, 2048, 2048,
                      4096, 4096, 3936, 3936):
                nc.gpsimd.dma_start(skt_t[:, o:o + w], skt[:, o:o + w])
                o += w
            assert o == WK

            m_ps = [ps_m.tile([128, L1_COLS], F32, name=f"m_ps{s}", tag=f"m_ps{s}")
                    for s in range(4)]
            c_ps = ps_c.tile([2, L1_COLS], F32)

            # csum matmuls run once per GROUP of 4 key tiles: the idle DVE
            # pre-accumulates the 4 exp(S) tiles so the tensor engine pays
            # one ones-contraction per group instead of four.
            GRP = 4
            p_acc = None
            for kt in range(NKT):
                kk = min(128, WK - kt * 128)
                r0 = kt * 128
                j = kt % GRP
                sv_t = svpool.tile([128, VE], F32R)
                nc.sync.dma_start(sv_t[:kk, :], svte[r0:r0 + kk, :])
                s_ps = ps_s.tile([128, L1_COLS], F32)
                nc.tensor.matmul(s_ps[:kk, :], skt_t[:, r0:r0 + kk], q1_t[:],
                                 start=True, stop=True)
                p_t = ppool.tile([128, L1_COLS], F32R)
                nc.scalar.activation(p_t[:kk, :], s_ps[:kk, :], EXP,
                                     scale=INV_SQRT_C)
                for s in range(4):
                    nc.tensor.matmul(m_ps[s][:], sv_t[:kk, 2 + 128 * s:2 + 128 * (s + 1)],
                                     p_t[:kk, :],
                                     start=(kt == 0), stop=(kt == NKT - 1))
                if j == 0:
                    p_prev = p_t
                    sv_first = sv_t  # full 128 rows; its ones-columns feed csum
                elif j == 1:
                    p_acc = paccpool.tile([128, L1_COLS], F32R)
                    nc.vector.tensor_add(p_acc[:kk, :], p_prev[:kk, :], p_t[:kk, :])
                else:
                    nc.vector.tensor_add(p_acc[:kk, :], p_acc[:kk, :], p_t[:kk, :])
                if j == GRP - 1 or kt == NKT - 1:
                    nc.tensor.matmul(c_ps[:], sv_first[:, 0:2], p_acc[:, :],
                                     start=(kt < GRP), stop=(kt == NKT - 1))

            for s in range(4):
                m_sb = opool.tile([128, L1_COLS], F32, name=f"m_sb{s}", tag="m_sb")
                nc.vector.tensor_copy(m_sb[:], m_ps[s][:])
                nc.sync.dma_start(nv[128 * s:128 * (s + 1), :], m_sb[:])
            c_sb = opool.tile([2, L1_COLS], F32)
            nc.vector.tensor_copy(c_sb[:], c_ps[:])
            nc.sync.dma_start(csum[:], c_sb[:])
    nc.compile()
    return nc


def _build_stage2():
    nc = bacc.Bacc("TRN2", target_bir_lowering=False, debug=False, num_devices=8)
    mk = nc.dram_tensor("mk", [C, HW], F32R, kind="ExternalInput").ap()
    qq = nc.dram_tensor("qq", [C, L2_WIN], F32R, kind="ExternalInput").ap()
    nvte = nc.dram_tensor("nvte", [HW, VE], F32R, kind="ExternalInput").ap()
    cs1 = nc.dram_tensor("cs1", [HW, 1], F32, kind="ExternalInput").ap()
    out = nc.dram_tensor("out", [VC, L2_WIN], F32, kind="ExternalOutput").ap()

    with tile.TileContext(nc) as tc:
        with (
            tc.tile_pool(name="const", bufs=1) as cpool,
            tc.tile_pool(name="nvt", bufs=1) as nvpool,
            tc.tile_pool(name="small", bufs=4) as smpool,
            tc.tile_pool(name="p2", bufs=26) as p2pool,
            tc.tile_pool(name="ob", bufs=6) as obpool,
            tc.tile_pool(name="ps_s", bufs=2, space="PSUM") as ps_s,
            tc.tile_pool(name="ps_o", bufs=1, space="PSUM") as ps_o,
            tc.tile_pool(name="ps_c", bufs=2, space="PSUM") as ps_c,
        ):
            mk_t = cpool.tile([C, HW], F32R)
            nc.gpsimd.dma_start(mk_t[:], mk[:])
            qq_t = cpool.tile([C, L2_WIN], F32R)
            nc.sync.dma_start(qq_t[:, 0:512], qq[:, 0:512])
            nc.gpsimd.dma_start(qq_t[:, 512:L2_WIN], qq[:, 512:L2_WIN])

            # load newV tiles; normalize the value part (cols 2:) by the
            # stage-1 column sums, keep the ones-columns unscaled so they
            # still produce stage-2 column sums.
            nvtn = []
            for t in range(NK2T):
                kk = min(128, HW - t * 128)
                r0 = t * 128
                raw = smpool.tile([128, VE], F32R, tag="nvraw")
                nc.sync.dma_start(raw[:kk, :], nvte[r0:r0 + kk, :])
                cs_t = smpool.tile([128, 1], F32, tag="cs")
                nc.sync.dma_start(cs_t[:kk, :], cs1[r0:r0 + kk, :])
                rc_t = smpool.tile([128, 1], F32, tag="rc")
                nc.vector.reciprocal(rc_t[:kk, :], cs_t[:kk, :])
                nrm = nvpool.tile([128, VE], F32R, tag=f"nvtn{t}", name=f"nvtn{t}")
                nc.vector.tensor_scalar_mul(nrm[:kk, 2:], raw[:kk, 2:],
                                            rc_t[:kk, 0:1])
                nc.vector.tensor_copy(nrm[:kk, 0:2], raw[:kk, 0:2])
                nvtn.append(nrm)

            col = 0
            for chunk in L2_CHUNKS:
                p2 = []
                for t in range(NK2T):
                    kk = min(128, HW - t * 128)
                    s_ps = ps_s.tile([128, 512], F32)
                    nc.tensor.matmul(s_ps[:kk, :chunk],
                                     mk_t[:, t * 128:t * 128 + kk],
                                     qq_t[:, col:col + chunk],
                                     start=True, stop=True)
                    p_t = p2pool.tile([128, 512], F32R, tag="p2")
                    nc.scalar.activation(p_t[:kk, :chunk], s_ps[:kk, :chunk],
                                         EXP, scale=INV_SQRT_C)
                    p2.append(p_t)

                o_ps = [ps_o.tile([128, 512], F32, name=f"o_ps{v}", tag=f"o_ps{v}")
                        for v in range(4)]
                c_ps = ps_c.tile([2, 512], F32)
                for t in range(NK2T):
                    kk = min(128, HW - t * 128)
                    nc.tensor.matmul(c_ps[:, :chunk], nvtn[t][:kk, 0:2],
                                     p2[t][:kk, :chunk],
                                     start=(t == 0), stop=(t == NK2T - 1))
                    for v in range(4):
                        nc.tensor.matmul(o_ps[v][:, :chunk],
                                         nvtn[t][:kk, 2 + 128 * v:2 + 128 * (v + 1)],
                                         p2[t][:kk, :chunk],
                                         start=(t == 0), stop=(t == NK2T - 1))

                rc = smpool.tile([1, 512], F32, tag="rc2")
                nc.vector.reciprocal(rc[:, :chunk], c_ps[0:1, :chunk])
                bc = smpool.tile([128, 512], F32, tag="bc")
                nc.gpsimd.partition_broadcast(bc[:, :chunk], rc[:1, :chunk])
                # copy PSUM->SBUF first so the accumulator banks free up for
                # the next chunk before the (broadcast-gated) normalization
                obs = []
                for v in range(4):
                    ob = obpool.tile([128, 512], F32, name=f"ob{v}", tag="ob")
                    nc.vector.tensor_copy(ob[:, :chunk], o_ps[v][:, :chunk])
                    obs.append(ob)
                for v in range(4):
                    nc.vector.tensor_mul(obs[v][:, :chunk], obs[v][:, :chunk],
                                         bc[:, :chunk])
                    nc.sync.dma_start(out[128 * v:128 * (v + 1), col:col + chunk],
                                      obs[v][:, :chunk])
                col += chunk
    nc.compile()
    return nc


def _run_with_retry(build_key, builder, in_maps):
    """Run a launch; on a transient device failure retry, rebuilding the
    program (fresh jit identity) on the second failure."""
    last = None
    for attempt in range(3):
        if build_key not in _cache:
            _cache[build_key] = builder()
        try:
            return run_bass_kernel_spmd(_cache[build_key], in_maps,
                                        list(range(8)))
        except Exception as e:  # device wedge / transient axon failure
            last = e
            time.sleep(3.0)
            if attempt >= 1:
                _cache.pop(build_key, None)
    raise last


def kernel(query_q, query_k, support_k, support_v):
    query_q = np.ascontiguousarray(query_q, dtype=np.float32)
    query_k = np.ascontiguousarray(query_k, dtype=np.float32)
    support_k = np.ascontiguousarray(support_k, dtype=np.float32)
    support_v = np.ascontiguousarray(support_v, dtype=np.float32)

    # ---- host layout prep ----
    # skT[b]: [C, SF*HW];  svte[b]: [SF*HW, 2+VC] (2 ones-cols | sv.T)
    skt = support_k.transpose(0, 2, 1, 3, 4).reshape(B, C, WK)
    svte = np.empty((B, WK, VE), np.float32)
    svte[:, :, :2] = 1.0
    svte[:, :, 2:] = support_v.transpose(0, 1, 3, 4, 2).reshape(B, WK, VC)
    q1 = query_q[:, MID].reshape(B, C, HW)
    l1_maps = []
    for core in range(8):
        b, lane = divmod(core, 4)
        l1_maps.append({
            "skt": skt[b],
            "svte": svte[b],
            "q1": np.ascontiguousarray(q1[b][:, lane * L1_COLS:(lane + 1) * L1_COLS]),
        })
    res1 = _run_with_retry("l1", _build_stage1, l1_maps)
    r1 = res1.results

    # assemble newV^T (+ ones cols) and stage-1 column sums per batch
    nvte = np.empty((B, HW, VE), np.float32)
    nvte[:, :, :2] = 1.0
    cs1 = np.empty((B, HW, 1), np.float32)
    for core in range(8):
        b, lane = divmod(core, 4)
        sl = slice(lane * L1_COLS, (lane + 1) * L1_COLS)
        nvte[b][sl, 2:] = r1[core]["nv"].T
        cs1[b][sl, 0] = r1[core]["csum"][0]

    # ---- stage 2 ----
    mk = query_k[:, MID].reshape(B, C, HW)
    qq = query_q.transpose(0, 2, 1, 3, 4).reshape(B, C, Q2)
    wins = [0, L2_OWN, 2 * L2_OWN, Q2 - L2_WIN]
    l2_maps = []
    for core in range(8):
        b, lane = divmod(core, 4)
        w = wins[lane]
        l2_maps.append({
            "mk": mk[b],
            "qq": np.ascontiguousarray(qq[b][:, w:w + L2_WIN]),
            "nvte": nvte[b],
            "cs1": cs1[b],
        })
    res2 = _run_with_retry("l2", _build_stage2, l2_maps)
    r2 = res2.results
    _cache["last_exec_ns"] = [res1.exec_time_ns, res2.exec_time_ns]

    outv = np.empty((B, VC, Q2), np.float32)
    for core in range(8):
        b, lane = divmod(core, 4)
        w = wins[lane]
        lo = lane * L2_OWN - w
        outv[b][:, lane * L2_OWN:(lane + 1) * L2_OWN] = \
            r2[core]["out"][:, lo:lo + L2_OWN]

    # outv[b][vc, q2], q2 = f*HW + h*W + w  ->  [B, F, VC, H, W]
    return np.ascontiguousarray(
        outv.reshape(B, VC, FRAME, H, W).transpose(0, 2, 1, 3, 4))


# revision 27
# speedup vs baseline: 1.2441x; 1.0547x over previous
"""Trainium2 Bass kernel for the two-stage DAN/MoVe attention module.

Computation (per batch b, C=128 channels):
  Stage 1:  S  = skT.T @ q1 / sqrt(C);  P  = softmax_k(S);  newV = sv @ P
  Stage 2:  S2 = mK.T @ qq / sqrt(C);   P2 = softmax_k2(S2); out = newV @ P2

Sharding: 8 cores = 2 batches x 4 lanes. Stage 1 splits the 1600 query
columns 4 ways (400 each); stage 2 splits the 14400 frame-query columns
4 ways (3712-wide windows, 3600 owned). Two SPMD launches; the host
transposes stage-1 results between launches.

All big matmuls run in float32r (single-pass fp32 PE mode, ~1.5e-4 rel
err, 4x faster than fp32) with the value/key matrices as the stationary
operand and exp(S) as the long moving operand, so weight loads hide
under the previous matmul's stream. Softmax skips max-subtraction
(scores are ~N(0,1); exp cannot overflow). Column sums fall out of two
ones-columns prepended to the value matrices (an M=2 matmul per key
tile); normalization happens on-device via reciprocal + per-partition
scaling (stage 1 sums applied in stage 2) or partition-broadcast
multiply (stage 2 sums).
"""

import math
import time

import numpy as np

import concourse.bass as bass
import concourse.tile as tile
from concourse import bacc, mybir
from concourse.bass_utils import run_bass_kernel_spmd

F32 = mybir.dt.float32
F32R = mybir.dt.float32r
EXP = mybir.ActivationFunctionType.Exp

B, FRAME, SFRAME, C, VC, H, W = 2, 9, 15, 128, 512, 40, 40
HW = H * W                      # 1600
MID = FRAME // 2                # 4
WK = SFRAME * HW                # 24000 support keys
NKT = (WK + 127) // 128         # 188 key tiles (last = 64 rows)
Q2 = FRAME * HW                 # 14400 stage-2 query columns per batch
NK2T = (HW + 127) // 128        # 13 stage-2 key tiles (last = 64 rows)
VE = VC + 2                     # value matrices carry 2 ones-columns

L1_COLS = HW // 4               # 400 owned stage-1 columns per lane
L2_WIN = 3712                   # 29 * 128, per-lane stage-2 window
L2_OWN = Q2 // 4                # 3600 owned columns
L2_CHUNKS = [512] * 7 + [128]
INV_SQRT_C = 1.0 / math.sqrt(C)

_cache = {}


def _build_stage1():
    nc = bacc.Bacc("TRN2", target_bir_lowering=False, debug=False, num_devices=8)
    skt = nc.dram_tensor("skt", [C, WK], F32R, kind="ExternalInput").ap()
    svte = nc.dram_tensor("svte", [WK, VE], F32R, kind="ExternalInput").ap()
    q1 = nc.dram_tensor("q1", [C, L1_COLS], F32R, kind="ExternalInput").ap()
    nv = nc.dram_tensor("nv", [VC, L1_COLS], F32, kind="ExternalOutput").ap()
    csum = nc.dram_tensor("csum", [2, L1_COLS], F32, kind="ExternalOutput").ap()

    with tile.TileContext(nc) as tc:
        with (
            tc.tile_pool(name="const", bufs=1) as cpool,
            tc.tile_pool(name="skt", bufs=8) as skpool,
            tc.tile_pool(name="svt", bufs=10) as svpool,
            tc.tile_pool(name="p", bufs=6) as ppool,
            tc.tile_pool(name="pacc", bufs=3) as paccpool,
            tc.tile_pool(name="out", bufs=5) as opool,
            tc.tile_pool(name="ps_s", bufs=2, space="PSUM") as ps_s,
            tc.tile_pool(name="ps_m", bufs=1, space="PSUM") as ps_m,
            tc.tile_pool(name="ps_c", bufs=1, space="PSUM") as ps_c,
        ):
            q1_t = cpool.tile([C, L1_COLS], F32R)
            nc.sync.dma_start(q1_t[:], q1[:])



            m_ps = [ps_m.tile([128, L1_COLS], F32, name=f"m_ps{s}", tag=f"m_ps{s}")
                    for s in range(4)]
            c_ps = ps_c.tile([2, L1_COLS], F32)

            # csum matmuls run once per GROUP of 4 key tiles: the idle DVE
            # pre-accumulates the 4 exp(S) tiles so the tensor engine pays
            # one ones-contraction per group instead of four.
            GRP = 4
            p_acc = None
            for kt in range(NKT):
                kk = min(128, WK - kt * 128)
                r0 = kt * 128
                j = kt % GRP
                # skT streamed just-in-time on the gpsimd (SWDGE) queues so
                # the svte stream on the sync (HWDGE) queues never waits
                # behind bulk loads; demand stays under the per-core HBM rate.
                sk_t = skpool.tile([C, 128], F32R, tag="sk")
                nc.gpsimd.dma_start(sk_t[:, :kk], skt[:, r0:r0 + kk])
                sv_t = svpool.tile([128, VE], F32R)
                nc.sync.dma_start(sv_t[:kk, :], svte[r0:r0 + kk, :])
                s_ps = ps_s.tile([128, L1_COLS], F32)
                nc.tensor.matmul(s_ps[:kk, :], sk_t[:, :kk], q1_t[:],
                                 start=True, stop=True)
                p_t = ppool.tile([128, L1_COLS], F32R)
                nc.scalar.activation(p_t[:kk, :], s_ps[:kk, :], EXP,
                                     scale=INV_SQRT_C)
                for s in range(4):
                    nc.tensor.matmul(m_ps[s][:], sv_t[:kk, 2 + 128 * s:2 + 128 * (s + 1)],
                                     p_t[:kk, :],
                                     start=(kt == 0), stop=(kt == NKT - 1))
                if j == 0:
                    p_prev = p_t
                    sv_first = sv_t  # full 128 rows; its ones-columns feed csum
                elif j == 1:
                    p_acc = paccpool.tile([128, L1_COLS], F32R)
                    nc.vector.tensor_add(p_acc[:kk, :], p_prev[:kk, :], p_t[:kk, :])
                else:
                    nc.vector.tensor_add(p_acc[:kk, :], p_acc[:kk, :], p_t[:kk, :])
                if j == GRP - 1 or kt == NKT - 1:
                    nc.tensor.matmul(c_ps[:], sv_first[:, 0:2], p_acc[:, :],
                                     start=(kt < GRP), stop=(kt == NKT - 1))

            for s in range(4):
                m_sb = opool.tile([128, L1_COLS], F32, name=f"m_sb{s}", tag="m_sb")
                nc.vector.tensor_copy(m_sb[:], m_ps[s][:])
                nc.sync.dma_start(nv[128 * s:128 * (s + 1), :], m_sb[:])
            c_sb = opool.tile([2, L1_COLS], F32)
            nc.vector.tensor_copy(c_sb[:], c_ps[:])
            nc.sync.dma_start(csum[:], c_sb[:])
    nc.compile()
    return nc


def _build_stage2():
    nc = bacc.Bacc("TRN2", target_bir_lowering=False, debug=False, num_devices=8)
    mk = nc.dram_tensor("mk", [C, HW], F32R, kind="ExternalInput").ap()
    qq = nc.dram_tensor("qq", [C, L2_WIN], F32R, kind="ExternalInput").ap()
    nvte = nc.dram_tensor("nvte", [HW, VE], F32R, kind="ExternalInput").ap()
    cs1 = nc.dram_tensor("cs1", [HW, 1], F32, kind="ExternalInput").ap()
    out = nc.dram_tensor("out", [VC, L2_WIN], F32, kind="ExternalOutput").ap()

    with tile.TileContext(nc) as tc:
        with (
            tc.tile_pool(name="const", bufs=1) as cpool,
            tc.tile_pool(name="nvt", bufs=1) as nvpool,
            tc.tile_pool(name="small", bufs=4) as smpool,
            tc.tile_pool(name="p2", bufs=26) as p2pool,
            tc.tile_pool(name="ob", bufs=6) as obpool,
            tc.tile_pool(name="ps_s", bufs=2, space="PSUM") as ps_s,
            tc.tile_pool(name="ps_o", bufs=1, space="PSUM") as ps_o,
            tc.tile_pool(name="ps_c", bufs=2, space="PSUM") as ps_c,
        ):
            mk_t = cpool.tile([C, HW], F32R)
            nc.gpsimd.dma_start(mk_t[:], mk[:])
            qq_t = cpool.tile([C, L2_WIN], F32R)
            nc.sync.dma_start(qq_t[:, 0:512], qq[:, 0:512])
            nc.gpsimd.dma_start(qq_t[:, 512:L2_WIN], qq[:, 512:L2_WIN])

            # load newV tiles; normalize the value part (cols 2:) by the
            # stage-1 column sums, keep the ones-columns unscaled so they
            # still produce stage-2 column sums.
            nvtn = []
            for t in range(NK2T):
                kk = min(128, HW - t * 128)
                r0 = t * 128
                raw = smpool.tile([128, VE], F32R, tag="nvraw")
                nc.sync.dma_start(raw[:kk, :], nvte[r0:r0 + kk, :])
                cs_t = smpool.tile([128, 1], F32, tag="cs")
                nc.sync.dma_start(cs_t[:kk, :], cs1[r0:r0 + kk, :])
                rc_t = smpool.tile([128, 1], F32, tag="rc")
                nc.vector.reciprocal(rc_t[:kk, :], cs_t[:kk, :])
                nrm = nvpool.tile([128, VE], F32R, tag=f"nvtn{t}", name=f"nvtn{t}")
                nc.vector.tensor_scalar_mul(nrm[:kk, 2:], raw[:kk, 2:],
                                            rc_t[:kk, 0:1])
                nc.vector.tensor_copy(nrm[:kk, 0:2], raw[:kk, 0:2])
                nvtn.append(nrm)

            col = 0
            for chunk in L2_CHUNKS:
                p2 = []
                for t in range(NK2T):
                    kk = min(128, HW - t * 128)
                    s_ps = ps_s.tile([128, 512], F32)
                    nc.tensor.matmul(s_ps[:kk, :chunk],
                                     mk_t[:, t * 128:t * 128 + kk],
                                     qq_t[:, col:col + chunk],
                                     start=True, stop=True)
                    p_t = p2pool.tile([128, 512], F32R, tag="p2")
                    nc.scalar.activation(p_t[:kk, :chunk], s_ps[:kk, :chunk],
                                         EXP, scale=INV_SQRT_C)
                    p2.append(p_t)

                o_ps = [ps_o.tile([128, 512], F32, name=f"o_ps{v}", tag=f"o_ps{v}")
                        for v in range(4)]
                c_ps = ps_c.tile([2, 512], F32)
                for t in range(NK2T):
                    kk = min(128, HW - t * 128)
                    nc.tensor.matmul(c_ps[:, :chunk], nvtn[t][:kk, 0:2],
                                     p2[t][:kk, :chunk],
                                     start=(t == 0), stop=(t == NK2T - 1))
                    for v in range(4):
                        nc.tensor.matmul(o_ps[v][:, :chunk],
                                         nvtn[t][:kk, 2 + 128 * v:2 + 128 * (v + 1)],
                                         p2[t][:kk, :chunk],
                                         start=(t == 0), stop=(t == NK2T - 1))

                rc = smpool.tile([1, 512], F32, tag="rc2")
                nc.vector.reciprocal(rc[:, :chunk], c_ps[0:1, :chunk])
                bc = smpool.tile([128, 512], F32, tag="bc")
                nc.gpsimd.partition_broadcast(bc[:, :chunk], rc[:1, :chunk])
                # copy PSUM->SBUF first so the accumulator banks free up for
                # the next chunk before the (broadcast-gated) normalization
                obs = []
                for v in range(4):
                    ob = obpool.tile([128, 512], F32, name=f"ob{v}", tag="ob")
                    nc.vector.tensor_copy(ob[:, :chunk], o_ps[v][:, :chunk])
                    obs.append(ob)
                for v in range(4):
                    nc.vector.tensor_mul(obs[v][:, :chunk], obs[v][:, :chunk],
                                         bc[:, :chunk])
                    nc.sync.dma_start(out[128 * v:128 * (v + 1), col:col + chunk],
                                      obs[v][:, :chunk])
                col += chunk
    nc.compile()
    return nc


def _run_with_retry(build_key, builder, in_maps):
    """Run a launch; on a transient device failure retry, rebuilding the
    program (fresh jit identity) on the second failure."""
    last = None
    for attempt in range(3):
        if build_key not in _cache:
            _cache[build_key] = builder()
        try:
            return run_bass_kernel_spmd(_cache[build_key], in_maps,
                                        list(range(8)))
        except Exception as e:  # device wedge / transient axon failure
            last = e
            time.sleep(3.0)
            if attempt >= 1:
                _cache.pop(build_key, None)
    raise last


def kernel(query_q, query_k, support_k, support_v):
    query_q = np.ascontiguousarray(query_q, dtype=np.float32)
    query_k = np.ascontiguousarray(query_k, dtype=np.float32)
    support_k = np.ascontiguousarray(support_k, dtype=np.float32)
    support_v = np.ascontiguousarray(support_v, dtype=np.float32)

    # ---- host layout prep ----
    # skT[b]: [C, SF*HW];  svte[b]: [SF*HW, 2+VC] (2 ones-cols | sv.T)
    skt = support_k.transpose(0, 2, 1, 3, 4).reshape(B, C, WK)
    svte = np.empty((B, WK, VE), np.float32)
    svte[:, :, :2] = 1.0
    svte[:, :, 2:] = support_v.transpose(0, 1, 3, 4, 2).reshape(B, WK, VC)
    q1 = query_q[:, MID].reshape(B, C, HW)
    l1_maps = []
    for core in range(8):
        b, lane = divmod(core, 4)
        l1_maps.append({
            "skt": skt[b],
            "svte": svte[b],
            "q1": np.ascontiguousarray(q1[b][:, lane * L1_COLS:(lane + 1) * L1_COLS]),
        })
    res1 = _run_with_retry("l1", _build_stage1, l1_maps)
    r1 = res1.results

    # assemble newV^T (+ ones cols) and stage-1 column sums per batch
    nvte = np.empty((B, HW, VE), np.float32)
    nvte[:, :, :2] = 1.0
    cs1 = np.empty((B, HW, 1), np.float32)
    for core in range(8):
        b, lane = divmod(core, 4)
        sl = slice(lane * L1_COLS, (lane + 1) * L1_COLS)
        nvte[b][sl, 2:] = r1[core]["nv"].T
        cs1[b][sl, 0] = r1[core]["csum"][0]

    # ---- stage 2 ----
    mk = query_k[:, MID].reshape(B, C, HW)
    qq = query_q.transpose(0, 2, 1, 3, 4).reshape(B, C, Q2)
    wins = [0, L2_OWN, 2 * L2_OWN, Q2 - L2_WIN]
    l2_maps = []
    for core in range(8):
        b, lane = divmod(core, 4)
        w = wins[lane]
        l2_maps.append({
            "mk": mk[b],
            "qq": np.ascontiguousarray(qq[b][:, w:w + L2_WIN]),
            "nvte": nvte[b],
            "cs1": cs1[b],
        })
    res2 = _run_with_retry("l2", _build_stage2, l2_maps)
    r2 = res2.results
    _cache["last_exec_ns"] = [res1.exec_time_ns, res2.exec_time_ns]

    outv = np.empty((B, VC, Q2), np.float32)
    for core in range(8):
        b, lane = divmod(core, 4)
        w = wins[lane]
        lo = lane * L2_OWN - w
        outv[b][:, lane * L2_OWN:(lane + 1) * L2_OWN] = \
            r2[core]["out"][:, lo:lo + L2_OWN]

    # outv[b][vc, q2], q2 = f*HW + h*W + w  ->  [B, F, VC, H, W]
    return np.ascontiguousarray(
        outv.reshape(B, VC, FRAME, H, W).transpose(0, 2, 1, 3, 4))


# revision 31
# speedup vs baseline: 1.2791x; 1.0281x over previous
"""Trainium2 Bass kernel for the two-stage DAN/MoVe attention module.

Computation (per batch b, C=128 channels):
  Stage 1:  S  = skT.T @ q1 / sqrt(C);  P  = softmax_k(S);  newV = sv @ P
  Stage 2:  S2 = mK.T @ qq / sqrt(C);   P2 = softmax_k2(S2); out = newV @ P2

Sharding: 8 cores = 2 batches x 4 lanes. Stage 1 splits the 1600 query
columns 4 ways (400 each); stage 2 splits the 14400 frame-query columns
4 ways (3712-wide windows, 3600 owned). Two SPMD launches; the host
transposes stage-1 results between launches.

All big matmuls run in float32r (single-pass fp32 PE mode, ~1.5e-4 rel
err, 4x faster than fp32) with the value/key matrices as the stationary
operand and exp(S) as the long moving operand, so weight loads hide
under the previous matmul's stream. Softmax skips max-subtraction
(scores are ~N(0,1); exp cannot overflow). Column sums fall out of two
ones-columns prepended to the value matrices (an M=2 matmul per key
tile); normalization happens on-device via reciprocal + per-partition
scaling (stage 1 sums applied in stage 2) or partition-broadcast
multiply (stage 2 sums).
"""

import math
import time

import numpy as np

import concourse.bass as bass
import concourse.tile as tile
from concourse import bacc, mybir
from concourse.bass_utils import run_bass_kernel_spmd

F32 = mybir.dt.float32
F32R = mybir.dt.float32r
EXP = mybir.ActivationFunctionType.Exp

B, FRAME, SFRAME, C, VC, H, W = 2, 9, 15, 128, 512, 40, 40
HW = H * W                      # 1600
MID = FRAME // 2                # 4
WK = SFRAME * HW                # 24000 support keys
NKT = (WK + 127) // 128         # 188 key tiles (last = 64 rows)
Q2 = FRAME * HW                 # 14400 stage-2 query columns per batch
NK2T = (HW + 127) // 128        # 13 stage-2 key tiles (last = 64 rows)
VE = VC + 2                     # value matrices carry 2 ones-columns

L1_COLS = HW // 4               # 400 owned stage-1 columns per lane
L2_WIN = 3712                   # 29 * 128, per-lane stage-2 window
L2_OWN = Q2 // 4                # 3600 owned columns
L2_CHUNKS = [512] * 7 + [128]
INV_SQRT_C = 1.0 / math.sqrt(C)

_cache = {}


FW = VE + 128                   # fused per-key-tile row: [svte row | skT col tile]


def _build_stage1():
    nc = bacc.Bacc("TRN2", target_bir_lowering=False, debug=False, num_devices=8)
    fus = nc.dram_tensor("fus", [NKT, 128, FW], F32R, kind="ExternalInput").ap()
    q1 = nc.dram_tensor("q1", [C, L1_COLS], F32R, kind="ExternalInput").ap()
    nv = nc.dram_tensor("nv", [VC, L1_COLS], F32, kind="ExternalOutput").ap()
    csum = nc.dram_tensor("csum", [2, L1_COLS], F32, kind="ExternalOutput").ap()

    with tile.TileContext(nc) as tc:
        with (
            tc.tile_pool(name="const", bufs=1) as cpool,
            tc.tile_pool(name="svt", bufs=10) as svpool,
            tc.tile_pool(name="p", bufs=6) as ppool,
            tc.tile_pool(name="pacc", bufs=3) as paccpool,
            tc.tile_pool(name="out", bufs=5) as opool,
            tc.tile_pool(name="ps_s", bufs=2, space="PSUM") as ps_s,
            tc.tile_pool(name="ps_m", bufs=1, space="PSUM") as ps_m,
            tc.tile_pool(name="ps_c", bufs=1, space="PSUM") as ps_c,
        ):
            q1_t = cpool.tile([C, L1_COLS], F32R)
            nc.sync.dma_start(q1_t[:], q1[:])



            m_ps = [ps_m.tile([128, L1_COLS], F32, name=f"m_ps{s}", tag=f"m_ps{s}")
                    for s in range(4)]
            c_ps = ps_c.tile([2, L1_COLS], F32)

            # csum matmuls run once per GROUP of 4 key tiles: the idle DVE
            # pre-accumulates the 4 exp(S) tiles so the tensor engine pays
            # one ones-contraction per group instead of four.
            GRP = 4
            NG = NKT // GRP
            p_acc = None
            pend = None  # previous group's (ones_ap, acc) — csum deferred one
            for kt in range(NKT):
                kk = min(128, WK - kt * 128)
                j = kt % GRP
                # one fused DMA per key tile: svte rows + the skT column tile
                fu_t = svpool.tile([128, FW], F32R, tag="fu")
                nc.sync.dma_start(fu_t[:], fus[kt])
                sv_t = fu_t[:, 0:VE]
                sk_t = fu_t[:, VE:VE + kk]
                s_ps = ps_s.tile([128, L1_COLS], F32)
                nc.tensor.matmul(s_ps[:kk, :], sk_t[:], q1_t[:],
                                 start=True, stop=True)
                p_t = ppool.tile([128, L1_COLS], F32R)
                nc.scalar.activation(p_t[:kk, :], s_ps[:kk, :], EXP,
                                     scale=INV_SQRT_C)
                for s in range(4):
                    nc.tensor.matmul(m_ps[s][:], fu_t[:kk, 2 + 128 * s:2 + 128 * (s + 1)],
                                     p_t[:kk, :],
                                     start=(kt == 0), stop=(kt == NKT - 1))
                if j == 0:
                    if pend is not None:  # csum for the PREVIOUS group: by now
                        g = kt // GRP     # its DVE accumulation has finished
                        nc.tensor.matmul(c_ps[:], pend[0], pend[1][:, :],
                                         start=(g == 1), stop=False)
                    p_prev = p_t
                    sv_first = fu_t  # full 128 rows; ones-columns feed csum
                elif j == 1:
                    p_acc = paccpool.tile([128, L1_COLS], F32R)
                    nc.vector.tensor_add(p_acc[:kk, :], p_prev[:kk, :], p_t[:kk, :])
                else:
                    nc.vector.tensor_add(p_acc[:kk, :], p_acc[:kk, :], p_t[:kk, :])
                if j == GRP - 1:
                    pend = (sv_first[:, 0:2], p_acc)
            nc.tensor.matmul(c_ps[:], pend[0], pend[1][:, :],
                             start=False, stop=True)

            for s in range(4):
                m_sb = opool.tile([128, L1_COLS], F32, name=f"m_sb{s}", tag="m_sb")
                nc.vector.tensor_copy(m_sb[:], m_ps[s][:])
                nc.sync.dma_start(nv[128 * s:128 * (s + 1), :], m_sb[:])
            c_sb = opool.tile([2, L1_COLS], F32)
            nc.vector.tensor_copy(c_sb[:], c_ps[:])
            nc.sync.dma_start(csum[:], c_sb[:])
    nc.compile()
    return nc


def _build_stage2():
    nc = bacc.Bacc("TRN2", target_bir_lowering=False, debug=False, num_devices=8)
    mk = nc.dram_tensor("mk", [C, HW], F32R, kind="ExternalInput").ap()
    qq = nc.dram_tensor("qq", [C, L2_WIN], F32R, kind="ExternalInput").ap()
    nvte = nc.dram_tensor("nvte", [HW, VE], F32R, kind="ExternalInput").ap()
    cs1 = nc.dram_tensor("cs1", [HW, 1], F32, kind="ExternalInput").ap()
    out = nc.dram_tensor("out", [VC, L2_WIN], F32, kind="ExternalOutput").ap()

    with tile.TileContext(nc) as tc:
        with (
            tc.tile_pool(name="const", bufs=1) as cpool,
            tc.tile_pool(name="nvt", bufs=1) as nvpool,
            tc.tile_pool(name="small", bufs=4) as smpool,
            tc.tile_pool(name="p2", bufs=26) as p2pool,
            tc.tile_pool(name="ob", bufs=6) as obpool,
            tc.tile_pool(name="ps_s", bufs=2, space="PSUM") as ps_s,
            tc.tile_pool(name="ps_o", bufs=1, space="PSUM") as ps_o,
            tc.tile_pool(name="ps_c", bufs=2, space="PSUM") as ps_c,
        ):
            mk_t = cpool.tile([C, HW], F32R)
            nc.gpsimd.dma_start(mk_t[:], mk[:])
            qq_t = cpool.tile([C, L2_WIN], F32R)
            nc.sync.dma_start(qq_t[:, 0:512], qq[:, 0:512])
            nc.gpsimd.dma_start(qq_t[:, 512:L2_WIN], qq[:, 512:L2_WIN])

            # load newV tiles; normalize the value part (cols 2:) by the
            # stage-1 column sums, keep the ones-columns unscaled so they
            # still produce stage-2 column sums.
            nvtn = []
            for t in range(NK2T):
                kk = min(128, HW - t * 128)
                r0 = t * 128
                raw = smpool.tile([128, VE], F32R, tag="nvraw")
                nc.sync.dma_start(raw[:kk, :], nvte[r0:r0 + kk, :])
                cs_t = smpool.tile([128, 1], F32, tag="cs")
                nc.sync.dma_start(cs_t[:kk, :], cs1[r0:r0 + kk, :])
                rc_t = smpool.tile([128, 1], F32, tag="rc")
                nc.vector.reciprocal(rc_t[:kk, :], cs_t[:kk, :])
                nrm = nvpool.tile([128, VE], F32R, tag=f"nvtn{t}", name=f"nvtn{t}")
                nc.vector.tensor_scalar_mul(nrm[:kk, 2:], raw[:kk, 2:],
                                            rc_t[:kk, 0:1])
                nc.vector.tensor_copy(nrm[:kk, 0:2], raw[:kk, 0:2])
                nvtn.append(nrm)

            col = 0
            for chunk in L2_CHUNKS:
                p2 = []
                for t in range(NK2T):
                    kk = min(128, HW - t * 128)
                    s_ps = ps_s.tile([128, 512], F32)
                    nc.tensor.matmul(s_ps[:kk, :chunk],
                                     mk_t[:, t * 128:t * 128 + kk],
                                     qq_t[:, col:col + chunk],
                                     start=True, stop=True)
                    p_t = p2pool.tile([128, 512], F32R, tag="p2")
                    nc.scalar.activation(p_t[:kk, :chunk], s_ps[:kk, :chunk],
                                         EXP, scale=INV_SQRT_C)
                    p2.append(p_t)

                o_ps = [ps_o.tile([128, 512], F32, name=f"o_ps{v}", tag=f"o_ps{v}")
                        for v in range(4)]
                c_ps = ps_c.tile([2, 512], F32)
                for t in range(NK2T):
                    kk = min(128, HW - t * 128)
                    nc.tensor.matmul(c_ps[:, :chunk], nvtn[t][:kk, 0:2],
                                     p2[t][:kk, :chunk],
                                     start=(t == 0), stop=(t == NK2T - 1))
                    for v in range(4):
                        nc.tensor.matmul(o_ps[v][:, :chunk],
                                         nvtn[t][:kk, 2 + 128 * v:2 + 128 * (v + 1)],
                                         p2[t][:kk, :chunk],
                                         start=(t == 0), stop=(t == NK2T - 1))

                rc = smpool.tile([1, 512], F32, tag="rc2")
                nc.vector.reciprocal(rc[:, :chunk], c_ps[0:1, :chunk])
                bc = smpool.tile([128, 512], F32, tag="bc")
                nc.gpsimd.partition_broadcast(bc[:, :chunk], rc[:1, :chunk])
                # copy PSUM->SBUF first so the accumulator banks free up for
                # the next chunk before the (broadcast-gated) normalization
                obs = []
                for v in range(4):
                    ob = obpool.tile([128, 512], F32, name=f"ob{v}", tag="ob")
                    nc.vector.tensor_copy(ob[:, :chunk], o_ps[v][:, :chunk])
                    obs.append(ob)
                for v in range(4):
                    nc.vector.tensor_mul(obs[v][:, :chunk], obs[v][:, :chunk],
                                         bc[:, :chunk])
                    nc.sync.dma_start(out[128 * v:128 * (v + 1), col:col + chunk],
                                      obs[v][:, :chunk])
                col += chunk
    nc.compile()
    return nc


def _run_with_retry(build_key, builder, in_maps):
    """Run a launch; on a transient device failure retry, rebuilding the
    program (fresh jit identity) on the second failure."""
    last = None
    for attempt in range(3):
        if build_key not in _cache:
            _cache[build_key] = builder()
        try:
            return run_bass_kernel_spmd(_cache[build_key], in_maps,
                                        list(range(8)))
        except Exception as e:  # device wedge / transient axon failure
            last = e
            time.sleep(3.0)
            if attempt >= 1:
                _cache.pop(build_key, None)
    raise last


def kernel(query_q, query_k, support_k, support_v):
    query_q = np.ascontiguousarray(query_q, dtype=np.float32)
    query_k = np.ascontiguousarray(query_k, dtype=np.float32)
    support_k = np.ascontiguousarray(support_k, dtype=np.float32)
    support_v = np.ascontiguousarray(support_v, dtype=np.float32)

    # ---- host layout prep ----
    # fused per-key-tile rows: [1, 1, sv.T row (VC) | skT column tile (128)]
    WKP = NKT * 128
    fus = np.zeros((B, NKT, 128, FW), np.float32)
    fus[:, :, :, 0:2] = 1.0
    svt_pad = np.zeros((B, WKP, VC), np.float32)
    svt_pad[:, :WK] = support_v.transpose(0, 1, 3, 4, 2).reshape(B, WK, VC)
    fus[:, :, :, 2:VE] = svt_pad.reshape(B, NKT, 128, VC)
    skt_pad = np.zeros((B, C, WKP), np.float32)
    skt_pad[:, :, :WK] = support_k.transpose(0, 2, 1, 3, 4).reshape(B, C, WK)
    fus[:, :, :, VE:] = skt_pad.reshape(B, C, NKT, 128).transpose(0, 2, 1, 3)
    q1 = query_q[:, MID].reshape(B, C, HW)
    l1_maps = []
    for core in range(8):
        b, lane = divmod(core, 4)
        l1_maps.append({
            "fus": fus[b],
            "q1": np.ascontiguousarray(q1[b][:, lane * L1_COLS:(lane + 1) * L1_COLS]),
        })
    res1 = _run_with_retry("l1", _build_stage1, l1_maps)
    r1 = res1.results

    # assemble newV^T (+ ones cols) and stage-1 column sums per batch
    nvte = np.empty((B, HW, VE), np.float32)
    nvte[:, :, :2] = 1.0
    cs1 = np.empty((B, HW, 1), np.float32)
    for core in range(8):
        b, lane = divmod(core, 4)
        sl = slice(lane * L1_COLS, (lane + 1) * L1_COLS)
        nvte[b][sl, 2:] = r1[core]["nv"].T
        cs1[b][sl, 0] = r1[core]["csum"][0]

    # ---- stage 2 ----
    mk = query_k[:, MID].reshape(B, C, HW)
    qq = query_q.transpose(0, 2, 1, 3, 4).reshape(B, C, Q2)
    wins = [0, L2_OWN, 2 * L2_OWN, Q2 - L2_WIN]
    l2_maps = []
    for core in range(8):
        b, lane = divmod(core, 4)
        w = wins[lane]
        l2_maps.append({
            "mk": mk[b],
            "qq": np.ascontiguousarray(qq[b][:, w:w + L2_WIN]),
            "nvte": nvte[b],
            "cs1": cs1[b],
        })
    res2 = _run_with_retry("l2", _build_stage2, l2_maps)
    r2 = res2.results
    _cache["last_exec_ns"] = [res1.exec_time_ns, res2.exec_time_ns]

    outv = np.empty((B, VC, Q2), np.float32)
    for core in range(8):
        b, lane = divmod(core, 4)
        w = wins[lane]
        lo = lane * L2_OWN - w
        outv[b][:, lane * L2_OWN:(lane + 1) * L2_OWN] = \
            r2[core]["out"][:, lo:lo + L2_OWN]

    # outv[b][vc, q2], q2 = f*HW + h*W + w  ->  [B, F, VC, H, W]
    return np.ascontiguousarray(
        outv.reshape(B, VC, FRAME, H, W).transpose(0, 2, 1, 3, 4))
